# revision 7
# baseline (speedup 1.0000x reference)
"""Reverse-time forget-mult recurrence on 8 Trainium2 NeuronCores.

h_t = f_t*x_t + (1-f_t)*h_{t+1}, h_{T+1}=0, over [T=2048, B=16, D=1024].

Strategy: shard D across the 8 cores (128 channels each) — the recurrence is
elementwise over (B, D), sequential only in T, so no cross-core communication.
On the host, each core's shard is laid out partition-major as [D_shard=128,
B=16, T] with the T axis reversed, so each (d, b) lane's full time series is
contiguous and the device scans forward.

All I/O is fp16 (inputs downconverted on the host, output upconverted): the
tensor_tensor_scan state is fp32 internally regardless of operand dtype and
the recurrence is a convex combination (contracting), so fp16 rounding stays
~1e-3 vs the fp32 reference. HBM traffic: 24 MiB per core.

Engine budget per core (measured): the DVE scan runs at II=2 (4.42 us per
2048-elem block, dtype-independent) and is the only scan-capable engine, so
DVE's 16 block-scans are the 71 us critical path. Everything else is kept off
it as far as possible: a = 1-f runs on the Scalar (Act) engine; g = f*x for
eight blocks runs on GpSimd (4 ns/elem) and stays on DVE (2x fp16 mode,
0.6 ns/elem) for the other eight, balancing GpSimd's slower rate against
DVE's scan load. Loads stream on the Sync HWDGE ring, stores on the Scalar
ring. Block 0 is loaded/activated/multiplied/scanned in quarter-T chunks so
the first scan starts ~5 us in instead of ~19; block 15 is scanned/stored in
chained quarter chunks to shorten the drain; blocks 0-1 stores are deferred
to the tail on the then-idle Sync ring.
"""

import numpy as np

T, B, D = 2048, 16, 1024
NCORES = 8
DS = D // NCORES          # 128 channels per core -> the SBUF partition dim
NBLK = B                  # 16 blocks of [128, T] per core
RB = 2                    # row-blocks per steady-state DMA (1 MiB transfers)
PB = 128
Q = T // 4

# blocks whose g = f*x runs on GpSimd (the rest run on DVE between scans)
GPS_BLOCKS = frozenset({2, 4, 6, 8, 10, 12, 14, 15})

_cached = {}


def _build():
    import concourse.bacc as bacc
    import concourse.mybir as mybir
    import concourse.tile as tile

    f16 = mybir.dt.float16
    M, A = mybir.AluOpType.mult, mybir.AluOpType.add
    Copy = mybir.ActivationFunctionType.Copy
    nc = bacc.Bacc("TRN2", target_bir_lowering=False, debug=False, num_devices=NCORES)
    f_in = nc.dram_tensor("f_in", [PB, NBLK, T], f16, kind="ExternalInput").ap()
    x_in = nc.dram_tensor("x_in", [PB, NBLK, T], f16, kind="ExternalInput").ap()
    h_out = nc.dram_tensor("h_out", [PB, NBLK, T], f16, kind="ExternalOutput").ap()

    with tile.TileContext(nc) as tc:
        with (
            tc.tile_pool(name="io", bufs=3) as io_pool,
            tc.tile_pool(name="b01", bufs=1) as b01_pool,
            tc.tile_pool(name="ap", bufs=2) as a_pool,
            tc.tile_pool(name="gp", bufs=4) as g_pool,
            tc.tile_pool(name="hp", bufs=4) as h_pool,
            tc.tile_pool(name="hd", bufs=1) as hd_pool,
        ):
            deferred = {}

            def compute_block(blk, f_ap, x_ap):
                """Emit a, g, scan (+store) for one full block given SBUF APs."""
                a_t = a_pool.tile([PB, T], f16, tag="a")
                nc.scalar.activation(a_t[:], f_ap, Copy, bias=1.0, scale=-1.0)
                g_t = g_pool.tile([PB, T], f16, tag="g")
                if blk in GPS_BLOCKS:
                    nc.gpsimd.tensor_mul(g_t[:], f_ap, x_ap)
                else:
                    nc.vector.tensor_mul(g_t[:], f_ap, x_ap)
                if blk <= 1:
                    h_t = hd_pool.tile([PB, T], f16, tag=f"hd{blk}", name=f"hd{blk}")
                else:
                    h_t = h_pool.tile([PB, T], f16, tag="h")
                if blk < NBLK - 1:
                    nc.vector.tensor_tensor_scan(h_t[:], a_t[:], g_t[:], 0.0, M, A)
                    if blk <= 1:
                        deferred[blk] = h_t
                    else:
                        nc.scalar.dma_start(out=h_out[:, blk, :], in_=h_t[:])
                else:
                    # last block: chained quarter-scans + quarter-stores to
                    # shorten the pipeline drain
                    for q in range(4):
                        qsl = slice(Q * q, Q * (q + 1))
                        init = 0.0 if q == 0 else h_t[:, Q * q - 1 : Q * q]
                        nc.vector.tensor_tensor_scan(
                            h_t[:, qsl], a_t[:, qsl], g_t[:, qsl], init, M, A
                        )
                        nc.scalar.dma_start(out=h_out[:, NBLK - 1, qsl], in_=h_t[:, qsl])

            # --- block 0: quarter-granularity fast start ------------------
            h0 = hd_pool.tile([PB, T], f16, tag="hd0", name="hd0")
            for q in range(4):
                qsl = slice(Q * q, Q * (q + 1))
                fq = b01_pool.tile([PB, Q], f16, tag=f"f0q{q}")
                nc.sync.dma_start(out=fq[:], in_=f_in[:, 0, qsl])
                xq = b01_pool.tile([PB, Q], f16, tag=f"x0q{q}")
                nc.sync.dma_start(out=xq[:], in_=x_in[:, 0, qsl])
                aq = b01_pool.tile([PB, Q], f16, tag=f"a0q{q}")
                nc.scalar.activation(aq[:], fq[:], Copy, bias=1.0, scale=-1.0)
                gq = b01_pool.tile([PB, Q], f16, tag=f"g0q{q}")
                nc.vector.tensor_mul(gq[:], fq[:], xq[:])
                init = 0.0 if q == 0 else h0[:, Q * q - 1 : Q * q]
                nc.vector.tensor_tensor_scan(h0[:, qsl], aq[:], gq[:], init, M, A)
            deferred[0] = h0

            # --- block 1: single-block load ------------------------------
            f1 = b01_pool.tile([PB, T], f16, tag="f1")
            nc.sync.dma_start(out=f1[:], in_=f_in[:, 1, :])
            x1 = b01_pool.tile([PB, T], f16, tag="x1")
            nc.sync.dma_start(out=x1[:], in_=x_in[:, 1, :])
            compute_block(1, f1[:], x1[:])

            # --- blocks 2..15: RB=2 steady state -------------------------
            for r in range(1, NBLK // RB):
                bsl = slice(RB * r, RB * (r + 1))
                f_t = io_pool.tile([PB, RB, T], f16, tag="f")
                nc.sync.dma_start(out=f_t[:], in_=f_in[:, bsl, :])
                x_t = io_pool.tile([PB, RB, T], f16, tag="x")
                nc.sync.dma_start(out=x_t[:], in_=x_in[:, bsl, :])
                if r == NBLK // RB - 1:
                    # the Sync ring is idle after the final load: flush the
                    # deferred block-0/1 stores there to fill the end DMA gap
                    for dblk, dh in deferred.items():
                        nc.sync.dma_start(out=h_out[:, dblk, :], in_=dh[:])
                for j in range(RB):
                    compute_block(RB * r + j, f_t[:, j, :], x_t[:, j, :])
    nc.compile()
    return nc


def _get_nc():
    if "nc" not in _cached:
        _cached["nc"] = _build()
    return _cached["nc"]


def _shard(arr):
    """[T, B, D] -> per-core fp16 [DS, B, T] (partition-major), T reversed."""
    v = arr[::-1].transpose(2, 1, 0)  # [D, B, T] strided view, T reversed
    return [
        v[DS * c : DS * (c + 1)].astype(np.float16) for c in range(NCORES)
    ]


def _run(f, x, trace=False):
    from concourse.bass_utils import run_bass_kernel_spmd

    f = np.asarray(f, dtype=np.float32)
    x = np.asarray(x, dtype=np.float32)
    assert f.shape == (T, B, D) and x.shape == (T, B, D)

    nc = _get_nc()
    f_shards = _shard(f)
    x_shards = _shard(x)
    in_maps = [{"f_in": f_shards[c], "x_in": x_shards[c]} for c in range(NCORES)]
    res = run_bass_kernel_spmd(nc, in_maps, core_ids=list(range(NCORES)), trace=trace)

    out = np.empty((T, B, D), dtype=np.float32)
    for c in range(NCORES):
        # h_c[d, b, t_rev] -> out[t, b, DS*c + d]
        out[:, :, DS * c : DS * (c + 1)] = res.results[c]["h_out"][:, :, ::-1].transpose(2, 1, 0)
    return out.reshape(T * B, D), res


def kernel(f, x):
    return _run(f, x, trace=False)[0]


# revision 8
# speedup vs baseline: 1.2103x; 1.2103x over previous
"""Reverse-time forget-mult recurrence on 8 Trainium2 NeuronCores.

h_t = f_t*x_t + (1-f_t)*h_{t+1}, h_{T+1}=0, over [T=2048, B=16, D=1024].

Strategy: shard D across the 8 cores (128 channels each) — the recurrence is
elementwise over (B, D), sequential only in T, so no cross-core communication.
On the host, each core's shard is laid out partition-major as [D_shard=128,
B=16, T] with the T axis reversed, so each (d, b) lane's full time series is
contiguous and the device scans forward.

All I/O is fp16 (inputs downconverted on the host, output upconverted): the
tensor_tensor_scan state is fp32 internally regardless of operand dtype and
the recurrence is a convex combination (contracting), so fp16 rounding stays
~1e-3 vs the fp32 reference. HBM traffic: 24 MiB per core.

Engine budget per core (measured): the DVE scan runs at II=2 (4.42 us per
2048-elem block, dtype-independent) and is the only scan-capable engine, so
DVE's 16 block-scans are the 71 us critical path. Everything else is kept off
it as far as possible: a = 1-f runs on the Scalar (Act) engine; g = f*x for
eight blocks runs on GpSimd (4 ns/elem) and stays on DVE (2x fp16 mode,
0.6 ns/elem) for the other eight, balancing GpSimd's slower rate against
DVE's scan load. Loads stream on the Sync HWDGE ring, stores on the Scalar
ring. Block 0 is loaded/activated/multiplied/scanned in quarter-T chunks so
the first scan starts ~5 us in instead of ~19; block 15 is scanned/stored in
chained quarter chunks to shorten the drain; blocks 0-1 stores are deferred
to the tail on the then-idle Sync ring.
"""

import numpy as np

T, B, D = 2048, 16, 1024
NCORES = 8
DS = D // NCORES          # 128 channels per core -> the SBUF partition dim
NBLK = B                  # 16 blocks of [128, T] per core
RB = 2                    # row-blocks per steady-state DMA (1 MiB transfers)
PB = 128
Q = T // 4

# blocks whose g = f*x runs on GpSimd. Empty: GpSimd SBUF traffic contends
# with the DVE (measured: concurrent gps muls slow DVE scans/muls 2-4x), so
# offloading to it is a net loss. All muls stay on the DVE.
GPS_BLOCKS = frozenset()

_cached = {}


def _build():
    import concourse.bacc as bacc
    import concourse.mybir as mybir
    import concourse.tile as tile

    f16 = mybir.dt.float16
    M, A = mybir.AluOpType.mult, mybir.AluOpType.add
    Copy = mybir.ActivationFunctionType.Copy
    nc = bacc.Bacc("TRN2", target_bir_lowering=False, debug=False, num_devices=NCORES)
    f_in = nc.dram_tensor("f_in", [PB, NBLK, T], f16, kind="ExternalInput").ap()
    x_in = nc.dram_tensor("x_in", [PB, NBLK, T], f16, kind="ExternalInput").ap()
    h_out = nc.dram_tensor("h_out", [PB, NBLK, T], f16, kind="ExternalOutput").ap()

    with tile.TileContext(nc) as tc:
        with (
            tc.tile_pool(name="io", bufs=3) as io_pool,
            tc.tile_pool(name="b01", bufs=1) as b01_pool,
            tc.tile_pool(name="ap", bufs=2) as a_pool,
            tc.tile_pool(name="gp", bufs=4) as g_pool,
            tc.tile_pool(name="hp", bufs=4) as h_pool,
            tc.tile_pool(name="hd", bufs=1) as hd_pool,
        ):
            deferred = {}

            def compute_block(blk, f_ap, x_ap):
                """Emit a, g, scan (+store) for one full block given SBUF APs."""
                a_t = a_pool.tile([PB, T], f16, tag="a")
                nc.scalar.activation(a_t[:], f_ap, Copy, bias=1.0, scale=-1.0)
                g_t = g_pool.tile([PB, T], f16, tag="g")
                if blk in GPS_BLOCKS:
                    nc.gpsimd.tensor_mul(g_t[:], f_ap, x_ap)
                else:
                    nc.vector.tensor_mul(g_t[:], f_ap, x_ap)
                if blk <= 1:
                    h_t = hd_pool.tile([PB, T], f16, tag=f"hd{blk}", name=f"hd{blk}")
                else:
                    h_t = h_pool.tile([PB, T], f16, tag="h")
                if blk < NBLK - 1:
                    nc.vector.tensor_tensor_scan(h_t[:], a_t[:], g_t[:], 0.0, M, A)
                    if blk <= 1:
                        deferred[blk] = h_t
                    else:
                        nc.scalar.dma_start(out=h_out[:, blk, :], in_=h_t[:])
                else:
                    # last block: chained quarter-scans + quarter-stores to
                    # shorten the pipeline drain
                    for q in range(4):
                        qsl = slice(Q * q, Q * (q + 1))
                        init = 0.0 if q == 0 else h_t[:, Q * q - 1 : Q * q]
                        nc.vector.tensor_tensor_scan(
                            h_t[:, qsl], a_t[:, qsl], g_t[:, qsl], init, M, A
                        )
                        nc.scalar.dma_start(out=h_out[:, NBLK - 1, qsl], in_=h_t[:, qsl])

            # --- block 0: quarter-granularity fast start ------------------
            h0 = hd_pool.tile([PB, T], f16, tag="hd0", name="hd0")
            for q in range(4):
                qsl = slice(Q * q, Q * (q + 1))
                fq = b01_pool.tile([PB, Q], f16, tag=f"f0q{q}")
                nc.sync.dma_start(out=fq[:], in_=f_in[:, 0, qsl])
                xq = b01_pool.tile([PB, Q], f16, tag=f"x0q{q}")
                nc.sync.dma_start(out=xq[:], in_=x_in[:, 0, qsl])
                aq = b01_pool.tile([PB, Q], f16, tag=f"a0q{q}")
                nc.scalar.activation(aq[:], fq[:], Copy, bias=1.0, scale=-1.0)
                gq = b01_pool.tile([PB, Q], f16, tag=f"g0q{q}")
                nc.vector.tensor_mul(gq[:], fq[:], xq[:])
                init = 0.0 if q == 0 else h0[:, Q * q - 1 : Q * q]
                nc.vector.tensor_tensor_scan(h0[:, qsl], aq[:], gq[:], init, M, A)
            deferred[0] = h0

            # --- block 1: single-block load ------------------------------
            f1 = b01_pool.tile([PB, T], f16, tag="f1")
            nc.sync.dma_start(out=f1[:], in_=f_in[:, 1, :])
            x1 = b01_pool.tile([PB, T], f16, tag="x1")
            nc.sync.dma_start(out=x1[:], in_=x_in[:, 1, :])
            compute_block(1, f1[:], x1[:])

            # --- blocks 2..15: RB=2 steady state -------------------------
            for r in range(1, NBLK // RB):
                bsl = slice(RB * r, RB * (r + 1))
                f_t = io_pool.tile([PB, RB, T], f16, tag="f")
                nc.sync.dma_start(out=f_t[:], in_=f_in[:, bsl, :])
                x_t = io_pool.tile([PB, RB, T], f16, tag="x")
                nc.sync.dma_start(out=x_t[:], in_=x_in[:, bsl, :])
                if r == NBLK // RB - 1:
                    # the Sync ring is idle after the final load: flush the
                    # deferred block-0/1 stores there to fill the end DMA gap
                    for dblk, dh in deferred.items():
                        nc.sync.dma_start(out=h_out[:, dblk, :], in_=dh[:])
                for j in range(RB):
                    compute_block(RB * r + j, f_t[:, j, :], x_t[:, j, :])
    nc.compile()
    return nc


def _get_nc():
    if "nc" not in _cached:
        _cached["nc"] = _build()
    return _cached["nc"]


def _shard(arr):
    """[T, B, D] -> per-core fp16 [DS, B, T] (partition-major), T reversed."""
    v = arr[::-1].transpose(2, 1, 0)  # [D, B, T] strided view, T reversed
    return [
        v[DS * c : DS * (c + 1)].astype(np.float16) for c in range(NCORES)
    ]


def _run(f, x, trace=False):
    from concourse.bass_utils import run_bass_kernel_spmd

    f = np.asarray(f, dtype=np.float32)
    x = np.asarray(x, dtype=np.float32)
    assert f.shape == (T, B, D) and x.shape == (T, B, D)

    nc = _get_nc()
    f_shards = _shard(f)
    x_shards = _shard(x)
    in_maps = [{"f_in": f_shards[c], "x_in": x_shards[c]} for c in range(NCORES)]
    res = run_bass_kernel_spmd(nc, in_maps, core_ids=list(range(NCORES)), trace=trace)

    out = np.empty((T, B, D), dtype=np.float32)
    for c in range(NCORES):
        # h_c[d, b, t_rev] -> out[t, b, DS*c + d]
        out[:, :, DS * c : DS * (c + 1)] = res.results[c]["h_out"][:, :, ::-1].transpose(2, 1, 0)
    return out.reshape(T * B, D), res


def kernel(f, x):
    return _run(f, x, trace=False)[0]
